# revision 10
# baseline (speedup 1.0000x reference)
"""MoE routing kernel for 8 Trainium2 NeuronCores.

Strategy (single fused launch; host handles routing + data movement):
  host   : router (fp32 gemm + sigmoid + top-2, selection-identical to
           jax.lax.top_k on this distribution), expert-sorted slot lists,
           gather + pre-score scaling + transpose into per-core [D, NTOT]
           streams, and the final post-score scaling + scatter-add combine.
  device : per core c = expert c. One dense GLU-MLP pipeline streaming
           column tiles: cols [0, 1024) are this core's shared-expert token
           slice, cols [1024, 1024+CAPE) are expert c's gathered slots.
           Shared weights load first in per-128-col blocks so the first
           matmul starts ~3us in; expert weights reuse the same SBUF blocks
           (per-block WAR deps let them stream in during the shared phase).
           Input DMAs issue on SP, output DMAs on Activation so a blocked
           input load never head-of-line-blocks an output store. All
           matmuls are fp32r at 1 cycle/row (tile widths even, >= 256); no
           transposes, no gathers, no collectives on device.
"""
import sys
sys.path.insert(0, '/opt/trn_rl_repo')

import numpy as np

import concourse.bacc as bacc
import concourse.mybir as mybir
import concourse.tile as tile
from concourse.bass_utils import run_bass_kernel_spmd

F32 = mybir.dt.float32
F32R = mybir.dt.float32r
AF = mybir.ActivationFunctionType

NCORES = 8
E = 8           # experts
K = 2           # top-k
D = 1024
H = 1024
T = 8192        # total tokens (B*S)
TPC = T // NCORES   # shared-expert tokens per core
ROUTE_SCALE = 1.0


def _expert_widths(cape):
    """Split cape columns into tiles of width 256..512.

    Widths must be even (fp32r matmul ISA restriction) and >= 256 so the
    cost of an fp32r matmul stays at 1 cycle/row.
    """
    cape = cape + (cape & 1)          # round up to even
    if cape <= 512:
        return [max(cape, 256)]
    k = cape // 512
    r = cape - 512 * k
    if r == 0:
        widths = [512] * k
    elif r >= 256:
        widths = [512] * k + [r]
    else:
        # borrow from one 512 so both trailing tiles stay >= 256; keep the
        # narrowest tile last to shorten the end-of-kernel drain chain
        widths = [512] * (k - 1) + [256 + r, 256]
    assert sum(widths) == cape and all(
        256 <= w <= 512 and w % 2 == 0 for w in widths), widths
    return widths


def build_moe(cape):
    widths = [512, 512] + _expert_widths(cape)   # shared tiles first
    starts = np.concatenate([[0], np.cumsum(widths)]).astype(int)
    ntiles = len(widths)
    ntot = int(starts[-1])

    nc = bacc.Bacc("TRN2", target_bir_lowering=False, debug=False,
                   num_devices=NCORES)
    xT = nc.dram_tensor("xT", [D, ntot], F32R, kind="ExternalInput").ap()
    w1T = nc.dram_tensor("w1T", [D, H], F32R, kind="ExternalInput").ap()
    w3T = nc.dram_tensor("w3T", [D, H], F32R, kind="ExternalInput").ap()
    w2T = nc.dram_tensor("w2T", [H, D], F32R, kind="ExternalInput").ap()
    sw1T = nc.dram_tensor("sw1T", [D, H], F32R, kind="ExternalInput").ap()
    sw3T = nc.dram_tensor("sw3T", [D, H], F32R, kind="ExternalInput").ap()
    sw2T = nc.dram_tensor("sw2T", [H, D], F32R, kind="ExternalInput").ap()
    yT_o = nc.dram_tensor("yT", [D, ntot], F32, kind="ExternalOutput").ap()

    with tile.TileContext(nc) as tc:
        with tc.tile_pool(name="pw", bufs=1) as pw, \
             tc.tile_pool(name="pxt", bufs=3) as pxt, \
             tc.tile_pool(name="pps", bufs=2, space="PSUM") as pps, \
             tc.tile_pool(name="pg", bufs=1) as pg, \
             tc.tile_pool(name="ps1", bufs=2) as ps1, \
             tc.tile_pool(name="pys", bufs=8) as pys:

            xTr = xT.rearrange("(k p) n -> p k n", p=128)

            def walloc(pfx):
                # per-128-col weight blocks: fine-grained WAR so the next
                # phase's loads stream in as each block's last reader retires
                return [pw.tile([128, 8, 128], F32R, tag=f"{pfx}_{m}",
                                name=f"{pfx}{m}")
                        for m in range(8)]

            def wblock(dst, srcT, m):
                nc.sync.dma_start(
                    dst[m][:],
                    srcT[:, m*128:(m+1)*128].rearrange("(k p) h -> p k h",
                                                       p=128))

            xts = {}

            def load_xt(t, split=False):
                tw = widths[t]
                tl = pxt.tile([128, 8, 512], F32R, tag="xt", name=f"xt{t}")
                if split:
                    nc.sync.dma_start(tl[:, 0:4, 0:tw],
                                      xTr[:, 0:4, starts[t]:starts[t]+tw])
                    return tl
                nc.sync.dma_start(tl[:, :, 0:tw],
                                  xTr[:, :, starts[t]:starts[t]+tw])
                xts[t] = tl
                return tl

            # --- prologue: shared weights (block loads) + first two x tiles
            w1 = walloc("w1")
            w3 = walloc("w3")
            w2 = walloc("w2")
            wblock(w1, sw1T, 0)
            xt0 = load_xt(0, split=True)          # k=0..3 first
            wblock(w3, sw3T, 0)
            nc.sync.dma_start(xt0[:, 4:8, 0:512], xTr[:, 4:8, 0:512])
            xts[0] = xt0
            for m in range(1, 8):
                wblock(w1, sw1T, m)
                wblock(w3, sw3T, m)
            load_xt(1)
            for m in range(8):
                wblock(w2, sw2T, m)
            load_xt(2)

            def do_tile(t, cw1, cw3, cw2):
                tw = widths[t]
                c0 = int(starts[t])
                gs = []
                for m in range(8):
                    h1 = pps.tile([128, 512], F32, tag="h1", name=f"h1_{t}_{m}")
                    h3 = pps.tile([128, 512], F32, tag="h3", name=f"h3_{t}_{m}")
                    for k in range(8):
                        nc.tensor.matmul(h1[:, 0:tw], cw1[m][:, k, :],
                                         xts[t][:, k, 0:tw],
                                         start=(k == 0), stop=(k == 7))
                    for k in range(8):
                        nc.tensor.matmul(h3[:, 0:tw], cw3[m][:, k, :],
                                         xts[t][:, k, 0:tw],
                                         start=(k == 0), stop=(k == 7))
                    s1 = ps1.tile([128, 512], F32, tag="s1", name=f"s1_{t}_{m}")
                    nc.scalar.activation(s1[:, 0:tw], h1[:, 0:tw], AF.Silu)
                    g = pg.tile([128, 512], F32R, tag=f"g{m}", name=f"g{t}_{m}")
                    nc.vector.tensor_mul(g[:, 0:tw], s1[:, 0:tw], h3[:, 0:tw])
                    gs.append(g)
                for d in range(8):
                    yp = pps.tile([128, 512], F32, tag="y", name=f"y_{t}_{d}")
                    # cw2 block d holds w2T[:, d*128:(d+1)*128] as
                    # [128 h-part, 8 h-chunk, 128 d-cols]
                    for m in range(8):
                        nc.tensor.matmul(yp[:, 0:tw], cw2[d][:, m, :],
                                         gs[m][:, 0:tw],
                                         start=(m == 0), stop=(m == 7))
                    ys = pys.tile([128, 512], F32, tag="ys", name=f"ys{t}_{d}")
                    nc.scalar.copy(ys[:, 0:tw], yp[:, 0:tw])
                    nc.scalar.dma_start(yT_o[d*128:(d+1)*128, c0:c0+tw],
                                        ys[:, 0:tw])

            # --- tile 0 (shared)
            do_tile(0, w1, w3, w2)

            # --- tile 1 start: prefetch xt3 + expert weights (reuse blocks)
            if ntiles > 3:
                load_xt(3)
            e1 = walloc("w1")
            e3 = walloc("w3")
            e2 = walloc("w2")
            for m in range(8):
                wblock(e1, w1T, m)
                wblock(e3, w3T, m)
            do_tile(1, w1, w3, w2)
            # expert w2 blocks queue after tile-1 issue; WAR frees per block
            for m in range(8):
                wblock(e2, w2T, m)

            # --- expert tiles (x prefetch runs 2 tiles ahead)
            for t in range(2, ntiles):
                if t + 2 < ntiles:
                    load_xt(t + 2)
                do_tile(t, e1, e3, e2)
    nc.compile()
    return nc


_BUILT = {}


def _get(name, builder, *args):
    key = (name,) + tuple(args)
    if key not in _BUILT:
        _BUILT[key] = builder(*args)
    return _BUILT[key], key


def kernel(**inputs):
    x = np.ascontiguousarray(np.asarray(inputs["x"], dtype=np.float32))
    xf = x.reshape(T, D)
    gw = np.asarray(inputs["gate_w"], dtype=np.float32)
    bias = np.asarray(inputs["expert_bias"], dtype=np.float32)
    w1 = np.asarray(inputs["w1"], dtype=np.float32)
    w2 = np.asarray(inputs["w2"], dtype=np.float32)
    w3 = np.asarray(inputs["w3"], dtype=np.float32)
    sw1 = np.asarray(inputs["sw1"], dtype=np.float32)
    sw2 = np.asarray(inputs["sw2"], dtype=np.float32)
    sw3 = np.asarray(inputs["sw3"], dtype=np.float32)
    cores = list(range(NCORES))

    # ---- router on host (exact: top-2 of sigmoid scores + bias) ----
    logits = xf @ gw.T
    scores = 1.0 / (1.0 + np.exp(-logits))
    sel = np.argsort(-(scores + bias[None, :]), axis=1, kind="stable")[:, :K]
    tops = (np.take_along_axis(scores, sel, axis=1) * ROUTE_SCALE)

    flat_sel = sel.reshape(-1)
    order = np.argsort(flat_sel, kind="stable")
    tok_idx = order // K
    s_sorted = tops.reshape(-1)[order]
    counts = np.bincount(flat_sel, minlength=E)
    offs = np.concatenate([[0], np.cumsum(counts)]).astype(int)
    cape = int(counts.max())

    nc, _ = _get("moe", build_moe, cape)
    ntot = 1024 + int(np.sum(_expert_widths(cape)))

    sw1T = np.ascontiguousarray(sw1.T)
    sw3T = np.ascontiguousarray(sw3.T)
    sw2T = np.ascontiguousarray(sw2.T)
    in_maps = []
    toks_c = []
    s_c = []
    for c in cores:
        n_c = int(counts[c])
        toks = tok_idx[offs[c]:offs[c] + n_c]
        s = s_sorted[offs[c]:offs[c] + n_c].astype(np.float32)
        toks_c.append(toks)
        s_c.append(s)
        xin = np.zeros((ntot, D), np.float32)
        xin[0:TPC] = xf[c*TPC:(c+1)*TPC]
        xin[1024:1024 + n_c] = xf[toks] * s[:, None]
        in_maps.append({
            "xT": np.ascontiguousarray(xin.T),
            "w1T": np.ascontiguousarray(w1[c].T),
            "w3T": np.ascontiguousarray(w3[c].T),
            "w2T": np.ascontiguousarray(w2[c].T),
            "sw1T": sw1T, "sw3T": sw3T, "sw2T": sw2T,
        })

    res = run_bass_kernel_spmd(nc, in_maps, cores).results

    # ---- combine on host ----
    out = np.empty((T, D), np.float32)
    for c in cores:
        yT = res[c]["yT"]
        out[c*TPC:(c+1)*TPC] = yT[:, 0:TPC].T
    for c in cores:
        yT = res[c]["yT"]
        n_c = int(counts[c])
        rows = yT[:, 1024:1024 + n_c].T * s_c[c][:, None]
        out[toks_c[c]] += rows
    return out.reshape(x.shape).astype(inputs["x"].dtype, copy=False)


# revision 13
# speedup vs baseline: 1.0142x; 1.0142x over previous
"""MoE routing kernel for 8 Trainium2 NeuronCores.

Strategy (single fused launch; host handles routing + data movement):
  host   : router (fp32 gemm + sigmoid + top-2, selection-identical to
           jax.lax.top_k on this distribution), expert-sorted slot lists,
           gather + pre-score scaling + transpose into per-core [D, NTOT]
           streams, and the final post-score scaling + scatter-add combine.
  device : per core c = expert c. One dense GLU-MLP pipeline streaming
           column tiles: cols [0, 1024) are this core's shared-expert token
           slice, cols [1024, 1024+CAPE) are expert c's gathered slots.
           Shared weights load first in per-128-col blocks so the first
           matmul starts ~3us in; expert weights reuse the same SBUF blocks
           (per-block WAR deps let them stream in during the shared phase).
           Input DMAs issue on SP, output DMAs on Activation so a blocked
           input load never head-of-line-blocks an output store. All
           matmuls are fp32r at 1 cycle/row (tile widths even, >= 256); no
           transposes, no gathers, no collectives on device.
"""
import sys
sys.path.insert(0, '/opt/trn_rl_repo')

import numpy as np

import concourse.bacc as bacc
import concourse.mybir as mybir
import concourse.tile as tile
from concourse.bass_utils import run_bass_kernel_spmd

F32 = mybir.dt.float32
F32R = mybir.dt.float32r
AF = mybir.ActivationFunctionType

NCORES = 8
E = 8           # experts
K = 2           # top-k
D = 1024
H = 1024
T = 8192        # total tokens (B*S)
TPC = T // NCORES   # shared-expert tokens per core
ROUTE_SCALE = 1.0


def _expert_widths(cape):
    """Split cape columns into tiles of width 256..512.

    Widths must be even (fp32r matmul ISA restriction) and >= 256 so the
    cost of an fp32r matmul stays at 1 cycle/row.
    """
    cape = cape + (cape & 1)          # round up to even
    if cape <= 512:
        return [max(cape, 256)]
    nt = -(-cape // 512)
    base = (cape // nt) & ~1
    rem = cape - base * nt            # leftover, even
    widths = [base + 2 * (1 if i < rem // 2 else 0) for i in range(nt)]
    widths[0] += cape - sum(widths)
    assert sum(widths) == cape and all(
        256 <= w <= 512 and w % 2 == 0 for w in widths), widths
    return widths


def build_moe(cape):
    widths = [512, 512] + _expert_widths(cape)   # shared tiles first
    starts = np.concatenate([[0], np.cumsum(widths)]).astype(int)
    ntiles = len(widths)
    ntot = int(starts[-1])

    nc = bacc.Bacc("TRN2", target_bir_lowering=False, debug=False,
                   num_devices=NCORES)
    xT = nc.dram_tensor("xT", [D, ntot], F32R, kind="ExternalInput").ap()
    w1T = nc.dram_tensor("w1T", [D, H], F32R, kind="ExternalInput").ap()
    w3T = nc.dram_tensor("w3T", [D, H], F32R, kind="ExternalInput").ap()
    w2T = nc.dram_tensor("w2T", [H, D], F32R, kind="ExternalInput").ap()
    sw1T = nc.dram_tensor("sw1T", [D, H], F32R, kind="ExternalInput").ap()
    sw3T = nc.dram_tensor("sw3T", [D, H], F32R, kind="ExternalInput").ap()
    sw2T = nc.dram_tensor("sw2T", [H, D], F32R, kind="ExternalInput").ap()
    yT_o = nc.dram_tensor("yT", [D, ntot], F32, kind="ExternalOutput").ap()

    with tile.TileContext(nc) as tc:
        with tc.tile_pool(name="pw", bufs=1) as pw, \
             tc.tile_pool(name="pxt", bufs=3) as pxt, \
             tc.tile_pool(name="pps", bufs=2, space="PSUM") as pps, \
             tc.tile_pool(name="pg", bufs=1) as pg, \
             tc.tile_pool(name="ps1", bufs=2) as ps1, \
             tc.tile_pool(name="pys", bufs=8) as pys:

            xTr = xT.rearrange("(k p) n -> p k n", p=128)

            def walloc(pfx):
                # per-128-col weight blocks: fine-grained WAR so the next
                # phase's loads stream in as each block's last reader retires
                return [pw.tile([128, 8, 128], F32R, tag=f"{pfx}_{m}",
                                name=f"{pfx}{m}")
                        for m in range(8)]

            def wblock(dst, srcT, m):
                nc.sync.dma_start(
                    dst[m][:],
                    srcT[:, m*128:(m+1)*128].rearrange("(k p) h -> p k h",
                                                       p=128))

            xts = {}

            def load_xt(t, split=False):
                tw = widths[t]
                tl = pxt.tile([128, 8, 512], F32R, tag="xt", name=f"xt{t}")
                if split:
                    nc.sync.dma_start(tl[:, 0:4, 0:tw],
                                      xTr[:, 0:4, starts[t]:starts[t]+tw])
                    return tl
                nc.sync.dma_start(tl[:, :, 0:tw],
                                  xTr[:, :, starts[t]:starts[t]+tw])
                xts[t] = tl
                return tl

            # --- prologue: shared weights (block loads) + first two x tiles
            w1 = walloc("w1")
            w3 = walloc("w3")
            w2 = walloc("w2")
            wblock(w1, sw1T, 0)
            xt0 = load_xt(0, split=True)          # k=0..3 first
            wblock(w3, sw3T, 0)
            nc.sync.dma_start(xt0[:, 4:8, 0:512], xTr[:, 4:8, 0:512])
            xts[0] = xt0
            for m in range(1, 8):
                wblock(w1, sw1T, m)
                wblock(w3, sw3T, m)
            load_xt(1)
            for m in range(8):
                wblock(w2, sw2T, m)
            load_xt(2)

            def do_tile(t, cw1, cw3, cw2):
                tw = widths[t]
                c0 = int(starts[t])
                gs = []
                for m in range(8):
                    h1 = pps.tile([128, 512], F32, tag="h1", name=f"h1_{t}_{m}",
                                  bufs=3)
                    h3 = pps.tile([128, 512], F32, tag="h3", name=f"h3_{t}_{m}",
                                  bufs=3)
                    for k in range(8):
                        nc.tensor.matmul(h1[:, 0:tw], cw1[m][:, k, :],
                                         xts[t][:, k, 0:tw],
                                         start=(k == 0), stop=(k == 7))
                    for k in range(8):
                        nc.tensor.matmul(h3[:, 0:tw], cw3[m][:, k, :],
                                         xts[t][:, k, 0:tw],
                                         start=(k == 0), stop=(k == 7))
                    s1 = ps1.tile([128, 512], F32, tag="s1", name=f"s1_{t}_{m}")
                    nc.scalar.activation(s1[:, 0:tw], h1[:, 0:tw], AF.Silu)
                    g = pg.tile([128, 512], F32R, tag=f"g{m}", name=f"g{t}_{m}")
                    nc.vector.tensor_mul(g[:, 0:tw], s1[:, 0:tw], h3[:, 0:tw])
                    gs.append(g)
                for d in range(8):
                    yp = pps.tile([128, 512], F32, tag="y", name=f"y_{t}_{d}")
                    # cw2 block d holds w2T[:, d*128:(d+1)*128] as
                    # [128 h-part, 8 h-chunk, 128 d-cols]
                    for m in range(8):
                        nc.tensor.matmul(yp[:, 0:tw], cw2[d][:, m, :],
                                         gs[m][:, 0:tw],
                                         start=(m == 0), stop=(m == 7))
                    ys = pys.tile([128, 512], F32, tag="ys", name=f"ys{t}_{d}")
                    nc.vector.tensor_copy(ys[:, 0:tw], yp[:, 0:tw])
                    nc.scalar.dma_start(yT_o[d*128:(d+1)*128, c0:c0+tw],
                                        ys[:, 0:tw])

            # --- tile 0 (shared)
            do_tile(0, w1, w3, w2)

            # --- tile 1 start: prefetch xt3 + expert weights (reuse blocks)
            if ntiles > 3:
                load_xt(3)
            e1 = walloc("w1")
            e3 = walloc("w3")
            e2 = walloc("w2")
            for m in range(8):
                wblock(e1, w1T, m)
                wblock(e3, w3T, m)
            do_tile(1, w1, w3, w2)
            # expert w2 blocks queue after tile-1 issue; WAR frees per block
            for m in range(8):
                wblock(e2, w2T, m)

            # --- expert tiles (x prefetch runs 2 tiles ahead)
            for t in range(2, ntiles):
                if t + 2 < ntiles:
                    load_xt(t + 2)
                do_tile(t, e1, e3, e2)
    nc.compile()
    return nc


_BUILT = {}


def _get(name, builder, *args):
    key = (name,) + tuple(args)
    if key not in _BUILT:
        _BUILT[key] = builder(*args)
    return _BUILT[key], key


def kernel(**inputs):
    x = np.ascontiguousarray(np.asarray(inputs["x"], dtype=np.float32))
    xf = x.reshape(T, D)
    gw = np.asarray(inputs["gate_w"], dtype=np.float32)
    bias = np.asarray(inputs["expert_bias"], dtype=np.float32)
    w1 = np.asarray(inputs["w1"], dtype=np.float32)
    w2 = np.asarray(inputs["w2"], dtype=np.float32)
    w3 = np.asarray(inputs["w3"], dtype=np.float32)
    sw1 = np.asarray(inputs["sw1"], dtype=np.float32)
    sw2 = np.asarray(inputs["sw2"], dtype=np.float32)
    sw3 = np.asarray(inputs["sw3"], dtype=np.float32)
    cores = list(range(NCORES))

    # ---- router on host (exact: top-2 of sigmoid scores + bias) ----
    logits = xf @ gw.T
    scores = 1.0 / (1.0 + np.exp(-logits))
    sel = np.argsort(-(scores + bias[None, :]), axis=1, kind="stable")[:, :K]
    tops = (np.take_along_axis(scores, sel, axis=1) * ROUTE_SCALE)

    flat_sel = sel.reshape(-1)
    order = np.argsort(flat_sel, kind="stable")
    tok_idx = order // K
    s_sorted = tops.reshape(-1)[order]
    counts = np.bincount(flat_sel, minlength=E)
    offs = np.concatenate([[0], np.cumsum(counts)]).astype(int)
    cape = int(counts.max())

    nc, _ = _get("moe", build_moe, cape)
    ntot = 1024 + int(np.sum(_expert_widths(cape)))

    sw1T = np.ascontiguousarray(sw1.T)
    sw3T = np.ascontiguousarray(sw3.T)
    sw2T = np.ascontiguousarray(sw2.T)
    in_maps = []
    toks_c = []
    s_c = []
    for c in cores:
        n_c = int(counts[c])
        toks = tok_idx[offs[c]:offs[c] + n_c]
        s = s_sorted[offs[c]:offs[c] + n_c].astype(np.float32)
        toks_c.append(toks)
        s_c.append(s)
        xin = np.zeros((ntot, D), np.float32)
        xin[0:TPC] = xf[c*TPC:(c+1)*TPC]
        xin[1024:1024 + n_c] = xf[toks] * s[:, None]
        in_maps.append({
            "xT": np.ascontiguousarray(xin.T),
            "w1T": np.ascontiguousarray(w1[c].T),
            "w3T": np.ascontiguousarray(w3[c].T),
            "w2T": np.ascontiguousarray(w2[c].T),
            "sw1T": sw1T, "sw3T": sw3T, "sw2T": sw2T,
        })

    res = run_bass_kernel_spmd(nc, in_maps, cores).results

    # ---- combine on host ----
    out = np.empty((T, D), np.float32)
    for c in cores:
        yT = res[c]["yT"]
        out[c*TPC:(c+1)*TPC] = yT[:, 0:TPC].T
    for c in cores:
        yT = res[c]["yT"]
        n_c = int(counts[c])
        rows = yT[:, 1024:1024 + n_c].T * s_c[c][:, None]
        out[toks_c[c]] += rows
    return out.reshape(x.shape).astype(inputs["x"].dtype, copy=False)


# revision 15
# speedup vs baseline: 1.0247x; 1.0103x over previous
"""MoE routing kernel for 8 Trainium2 NeuronCores.

Strategy (single fused launch; host handles routing + data movement):
  host   : router (fp32 gemm + sigmoid + top-2, selection-identical to
           jax.lax.top_k on this distribution), expert-sorted slot lists,
           gather + pre-score scaling + transpose into per-core [D, NTOT]
           streams, and the final post-score scaling + scatter-add combine.
  device : per core c = expert c. One dense GLU-MLP pipeline streaming
           column tiles: cols [0, 1024) are this core's shared-expert token
           slice, cols [1024, 1024+CAPE) are expert c's gathered slots.
           Shared weights load first in per-128-col blocks so the first
           matmul starts ~3us in; expert weights reuse the same SBUF blocks
           (per-block WAR deps let them stream in during the shared phase).
           Input DMAs issue on SP, output DMAs on Activation so a blocked
           input load never head-of-line-blocks an output store. All
           matmuls are fp32r at 1 cycle/row (tile widths even, >= 256); no
           transposes, no gathers, no collectives on device.
"""
import sys
sys.path.insert(0, '/opt/trn_rl_repo')

import numpy as np

import concourse.bacc as bacc
import concourse.mybir as mybir
import concourse.tile as tile
from concourse.bass_utils import run_bass_kernel_spmd

F32 = mybir.dt.float32
F32R = mybir.dt.float32r
AF = mybir.ActivationFunctionType

NCORES = 8
E = 8           # experts
K = 2           # top-k
D = 1024
H = 1024
T = 8192        # total tokens (B*S)
TPC = T // NCORES   # shared-expert tokens per core
ROUTE_SCALE = 1.0


def _expert_widths(cape):
    """Split cape columns into tiles of width 256..512.

    Widths must be even (fp32r matmul ISA restriction) and >= 256 so the
    cost of an fp32r matmul stays at 1 cycle/row.
    """
    cape = cape + (cape & 1)          # round up to even
    if cape <= 512:
        return [max(cape, 256)]
    nt = -(-cape // 512)
    base = (cape // nt) & ~1
    rem = cape - base * nt            # leftover, even
    widths = [base + 2 * (1 if i < rem // 2 else 0) for i in range(nt)]
    widths[0] += cape - sum(widths)
    assert sum(widths) == cape and all(
        256 <= w <= 512 and w % 2 == 0 for w in widths), widths
    return widths


def build_moe(cape):
    widths = [512, 512] + _expert_widths(cape)   # shared tiles first
    starts = np.concatenate([[0], np.cumsum(widths)]).astype(int)
    ntiles = len(widths)
    ntot = int(starts[-1])

    nc = bacc.Bacc("TRN2", target_bir_lowering=False, debug=False,
                   num_devices=NCORES)
    xT = nc.dram_tensor("xT", [D, ntot], F32R, kind="ExternalInput").ap()
    w1T = nc.dram_tensor("w1T", [D, H], F32R, kind="ExternalInput").ap()
    w3T = nc.dram_tensor("w3T", [D, H], F32R, kind="ExternalInput").ap()
    w2T = nc.dram_tensor("w2T", [H, D], F32R, kind="ExternalInput").ap()
    sw1T = nc.dram_tensor("sw1T", [D, H], F32R, kind="ExternalInput").ap()
    sw3T = nc.dram_tensor("sw3T", [D, H], F32R, kind="ExternalInput").ap()
    sw2T = nc.dram_tensor("sw2T", [H, D], F32R, kind="ExternalInput").ap()
    yT_o = nc.dram_tensor("yT", [D, ntot], F32, kind="ExternalOutput").ap()

    with tile.TileContext(nc) as tc:
        with tc.tile_pool(name="pw", bufs=1) as pw, \
             tc.tile_pool(name="pxt", bufs=3) as pxt, \
             tc.tile_pool(name="pps", bufs=2, space="PSUM") as pps, \
             tc.tile_pool(name="pg", bufs=1) as pg, \
             tc.tile_pool(name="ps1", bufs=2) as ps1, \
             tc.tile_pool(name="pys", bufs=8) as pys:

            xTr = xT.rearrange("(k p) n -> p k n", p=128)

            def walloc(pfx):
                # per-128-col weight blocks: fine-grained WAR so the next
                # phase's loads stream in as each block's last reader retires
                return [pw.tile([128, 8, 128], F32R, tag=f"{pfx}_{m}",
                                name=f"{pfx}{m}")
                        for m in range(8)]

            def wblock(dst, srcT, m):
                nc.sync.dma_start(
                    dst[m][:],
                    srcT[:, m*128:(m+1)*128].rearrange("(k p) h -> p k h",
                                                       p=128))

            xts = {}

            def load_xt(t, split=False):
                tw = widths[t]
                tl = pxt.tile([128, 8, 512], F32R, tag="xt", name=f"xt{t}")
                if split:
                    nc.sync.dma_start(tl[:, 0:4, 0:tw],
                                      xTr[:, 0:4, starts[t]:starts[t]+tw])
                    return tl
                nc.sync.dma_start(tl[:, :, 0:tw],
                                  xTr[:, :, starts[t]:starts[t]+tw])
                xts[t] = tl
                return tl

            # --- prologue: shared weights (block loads) + first two x tiles
            w1 = walloc("w1")
            w3 = walloc("w3")
            w2 = walloc("w2")
            wblock(w1, sw1T, 0)
            xt0 = load_xt(0, split=True)          # k=0..3 first
            wblock(w3, sw3T, 0)
            nc.sync.dma_start(xt0[:, 4:8, 0:512], xTr[:, 4:8, 0:512])
            xts[0] = xt0

            # --- PE warm-up: dummy matmuls on a zeroed scratch tile keep the
            # tensor engine continuously busy through the prologue DMAs, so
            # its p-state is at full clock when the first real matmul issues
            scr = pg.tile([128, 512], F32, tag="scr", name="scr")
            nc.vector.memset(scr[:], 0)
            scrr = scr.bitcast(F32R)
            for i in range(24):
                wp = pps.tile([128, 512], F32, tag="y", name=f"warm{i}")
                nc.tensor.matmul(wp[:], scrr[:, 0:128], scrr[:],
                                 start=True, stop=True)
            for m in range(1, 8):
                wblock(w1, sw1T, m)
                wblock(w3, sw3T, m)
            load_xt(1)
            for m in range(8):
                wblock(w2, sw2T, m)
            load_xt(2)

            def do_tile(t, cw1, cw3, cw2):
                tw = widths[t]
                c0 = int(starts[t])
                gs = []
                for m in range(8):
                    h1 = pps.tile([128, 512], F32, tag="h1", name=f"h1_{t}_{m}",
                                  bufs=3)
                    h3 = pps.tile([128, 512], F32, tag="h3", name=f"h3_{t}_{m}",
                                  bufs=3)
                    for k in range(8):
                        nc.tensor.matmul(h1[:, 0:tw], cw1[m][:, k, :],
                                         xts[t][:, k, 0:tw],
                                         start=(k == 0), stop=(k == 7))
                    for k in range(8):
                        nc.tensor.matmul(h3[:, 0:tw], cw3[m][:, k, :],
                                         xts[t][:, k, 0:tw],
                                         start=(k == 0), stop=(k == 7))
                    s1 = ps1.tile([128, 512], F32, tag="s1", name=f"s1_{t}_{m}")
                    nc.scalar.activation(s1[:, 0:tw], h1[:, 0:tw], AF.Silu)
                    g = pg.tile([128, 512], F32R, tag=f"g{m}", name=f"g{t}_{m}")
                    nc.vector.tensor_mul(g[:, 0:tw], s1[:, 0:tw], h3[:, 0:tw])
                    gs.append(g)
                for d in range(8):
                    yp = pps.tile([128, 512], F32, tag="y", name=f"y_{t}_{d}")
                    # cw2 block d holds w2T[:, d*128:(d+1)*128] as
                    # [128 h-part, 8 h-chunk, 128 d-cols]
                    for m in range(8):
                        nc.tensor.matmul(yp[:, 0:tw], cw2[d][:, m, :],
                                         gs[m][:, 0:tw],
                                         start=(m == 0), stop=(m == 7))
                    ys = pys.tile([128, 512], F32, tag="ys", name=f"ys{t}_{d}")
                    nc.vector.tensor_copy(ys[:, 0:tw], yp[:, 0:tw])
                    nc.scalar.dma_start(yT_o[d*128:(d+1)*128, c0:c0+tw],
                                        ys[:, 0:tw])

            # --- tile 0 (shared)
            do_tile(0, w1, w3, w2)

            # --- tile 1 start: prefetch xt3 + expert weights (reuse blocks)
            if ntiles > 3:
                load_xt(3)
            e1 = walloc("w1")
            e3 = walloc("w3")
            e2 = walloc("w2")
            for m in range(8):
                wblock(e1, w1T, m)
                wblock(e3, w3T, m)
            do_tile(1, w1, w3, w2)
            # expert w2 blocks queue after tile-1 issue; WAR frees per block
            for m in range(8):
                wblock(e2, w2T, m)

            # --- expert tiles (x prefetch runs 2 tiles ahead)
            for t in range(2, ntiles):
                if t + 2 < ntiles:
                    load_xt(t + 2)
                do_tile(t, e1, e3, e2)
    nc.compile()
    return nc


_BUILT = {}


def _get(name, builder, *args):
    key = (name,) + tuple(args)
    if key not in _BUILT:
        _BUILT[key] = builder(*args)
    return _BUILT[key], key


def kernel(**inputs):
    x = np.ascontiguousarray(np.asarray(inputs["x"], dtype=np.float32))
    xf = x.reshape(T, D)
    gw = np.asarray(inputs["gate_w"], dtype=np.float32)
    bias = np.asarray(inputs["expert_bias"], dtype=np.float32)
    w1 = np.asarray(inputs["w1"], dtype=np.float32)
    w2 = np.asarray(inputs["w2"], dtype=np.float32)
    w3 = np.asarray(inputs["w3"], dtype=np.float32)
    sw1 = np.asarray(inputs["sw1"], dtype=np.float32)
    sw2 = np.asarray(inputs["sw2"], dtype=np.float32)
    sw3 = np.asarray(inputs["sw3"], dtype=np.float32)
    cores = list(range(NCORES))

    # ---- router on host (exact: top-2 of sigmoid scores + bias) ----
    logits = xf @ gw.T
    scores = 1.0 / (1.0 + np.exp(-logits))
    sel = np.argsort(-(scores + bias[None, :]), axis=1, kind="stable")[:, :K]
    tops = (np.take_along_axis(scores, sel, axis=1) * ROUTE_SCALE)

    flat_sel = sel.reshape(-1)
    order = np.argsort(flat_sel, kind="stable")
    tok_idx = order // K
    s_sorted = tops.reshape(-1)[order]
    counts = np.bincount(flat_sel, minlength=E)
    offs = np.concatenate([[0], np.cumsum(counts)]).astype(int)
    cape = int(counts.max())

    nc, _ = _get("moe", build_moe, cape)
    ntot = 1024 + int(np.sum(_expert_widths(cape)))

    sw1T = np.ascontiguousarray(sw1.T)
    sw3T = np.ascontiguousarray(sw3.T)
    sw2T = np.ascontiguousarray(sw2.T)
    in_maps = []
    toks_c = []
    s_c = []
    for c in cores:
        n_c = int(counts[c])
        toks = tok_idx[offs[c]:offs[c] + n_c]
        s = s_sorted[offs[c]:offs[c] + n_c].astype(np.float32)
        toks_c.append(toks)
        s_c.append(s)
        xin = np.zeros((ntot, D), np.float32)
        xin[0:TPC] = xf[c*TPC:(c+1)*TPC]
        xin[1024:1024 + n_c] = xf[toks] * s[:, None]
        in_maps.append({
            "xT": np.ascontiguousarray(xin.T),
            "w1T": np.ascontiguousarray(w1[c].T),
            "w3T": np.ascontiguousarray(w3[c].T),
            "w2T": np.ascontiguousarray(w2[c].T),
            "sw1T": sw1T, "sw3T": sw3T, "sw2T": sw2T,
        })

    res = run_bass_kernel_spmd(nc, in_maps, cores).results

    # ---- combine on host ----
    out = np.empty((T, D), np.float32)
    for c in cores:
        yT = res[c]["yT"]
        out[c*TPC:(c+1)*TPC] = yT[:, 0:TPC].T
    for c in cores:
        yT = res[c]["yT"]
        n_c = int(counts[c])
        rows = yT[:, 1024:1024 + n_c].T * s_c[c][:, None]
        out[toks_c[c]] += rows
    return out.reshape(x.shape).astype(inputs["x"].dtype, copy=False)
